# revision 20
# baseline (speedup 1.0000x reference)
"""BaseAttentionPooling Trainium2 kernel.

reference:
    h = tanh(x @ W1 + b1)            # [N, H]
    logits = (h @ W2 + b2)[:, 0]     # [N]
    per-graph softmax over sorted `batch`, pooled = seg_sum(x * w)  # [G, D]

Strategy (data-parallel over graphs, 8 cores, SPMD-identical program):
  - 512 graphs/core, 8 blocks of 64 graphs; nodes padded to `cpb` chunks
    of 128 per block (cpb uniform across cores/blocks).
  - Host ships x node-major bf16 (pooling, accuracy-critical) plus a
    d-major fp8(e4m3) DoubleRow-interleaved copy (MLP input; logits
    tolerate fp8), merged per 8-chunk group into one DMA descriptor.
  - b2 dropped (cancels in softmax); exp without max-subtraction is safe
    because |logits| <= ||W2||_1 + |b2| is small.
  - PE per chunk: hT = W1.T @ xT (fp8 DoubleRow), logits via
    stationary-h matmul (1 col), pooled via stationary-oh matmul whose
    257-col moving x (augmented ones column) accumulates pooled AND den.
  - oh[i, g] = (iota[g] == rel_gid[i]) * e[i], built for a whole 8-chunk
    group in two DVE tensor_tensor ops (broadcast APs), all-bf16.
  - PSUM: four 64-graph block accumulators pack per [128, 257] f32 bank.
"""

import os
import sys

import numpy as np

for _p in ("/opt/trn_rl_repo",):
    if _p not in sys.path and os.path.isdir(_p):
        sys.path.insert(0, _p)

import ml_dtypes

import concourse.bass as bass
import concourse.tile as tile
from concourse import bacc, mybir
from concourse import bass_utils

N, D, H, G = 500000, 256, 128, 4096
NCORES = 8
GPC = G // NCORES          # graphs per core = 512
BLKG = 64                  # graphs per block
NBLK = GPC // BLKG         # blocks per core = 8
P = 128                    # partition / chunk size

BF16 = mybir.dt.bfloat16
F32 = mybir.dt.float32
F8 = mybir.dt.float8e4
NP_BF16 = ml_dtypes.bfloat16
NP_F8 = ml_dtypes.float8_e4m3fn

LAST_RESULT = None  # test.py reads exec_time_ns / profile from here


# ---------------------------------------------------------------- host plan

def make_plan(batch):
    """Compute the uniform chunk layout from the sorted graph ids."""
    batch = np.asarray(batch)
    seg = np.searchsorted(batch, np.arange(G + 1), side="left")  # [G+1]
    counts = np.zeros((NCORES, NBLK), dtype=np.int64)
    for c in range(NCORES):
        for b in range(NBLK):
            g0 = c * GPC + b * BLKG
            counts[c, b] = seg[g0 + BLKG] - seg[g0]
    cpb = int(np.ceil(counts.max() / P))
    cpb = ((cpb + 7) // 8) * 8     # whole number of 8-chunk groups
    ch = NBLK * cpb                # chunks per core
    return seg, cpb, ch


def build_inputs(x, batch, W1, b1, W2, seg, cpb, ch):
    """Build the 8 per-core input maps (layout/precision prep only)."""
    x = np.asarray(x)
    batch = np.asarray(batch)
    n_g8 = ch // 8

    w1_f = np.asarray(W1, dtype=np.float32)          # [256, 128]
    # DoubleRow k-tile interleave: w1dr[p, t*128+m] = W1[t*128+p, m]
    w1dr = np.ascontiguousarray(
        w1_f.reshape(2, P, H).transpose(1, 0, 2).reshape(P, 2 * H)
    ).astype(NP_F8)
    b1_f = np.asarray(b1, dtype=np.float32).reshape(H, 1)
    w2_bf = np.asarray(W2, dtype=np.float32).astype(NP_BF16).reshape(H, 1)
    # io8[p, j*64+g] = g  (iota tiled per chunk-of-group)
    io8 = np.broadcast_to(
        np.arange(BLKG, dtype=np.float32), (P, 8, BLKG)
    ).reshape(P, 8 * BLKG).astype(NP_BF16)

    in_maps = []
    for c in range(NCORES):
        xpad = np.zeros((ch * P, D), dtype=np.float32)
        rel = np.full(ch * P, -1.0, dtype=np.float32)
        for b in range(NBLK):
            g0 = c * GPC + b * BLKG
            s0, s1 = int(seg[g0]), int(seg[g0 + BLKG])
            n = s1 - s0
            r0 = b * cpb * P
            xpad[r0 : r0 + n] = x[s0:s1]
            rel[r0 : r0 + n] = (batch[s0:s1] - g0).astype(np.float32)
        # node-major bf16 with a ones-column appended per chunk (row
        # [x(256) | 1] so one 257-col matmul accumulates pooled AND den),
        # tiled so each 8-chunk group is one contiguous DRAM block
        xaug = np.ones((ch * P, D + 1), dtype=NP_BF16)
        xaug[:, :D] = xpad.astype(NP_BF16)
        xs_t = np.ascontiguousarray(
            xaug.reshape(n_g8, 8, P, D + 1)
            .transpose(0, 2, 1, 3)
            .reshape(n_g8 * P, 8 * (D + 1))
        )
        # d-major fp8 with DoubleRow interleave:
        # xt_t[g8*128+p, (t*8+j)*128 + i] = xpad[(g8*8+j)*128 + i, t*128+p]
        xt_t = np.ascontiguousarray(
            xpad.astype(NP_F8)
            .reshape(n_g8, 8, P, 2, P)   # [g8, j, i, t, p]
            .transpose(0, 4, 3, 1, 2)    # [g8, p, t, j, i]
            .reshape(n_g8 * P, 2 * 8 * P)
        )
        blr = np.ascontiguousarray(
            rel.reshape(ch, P).T.astype(NP_BF16)
        )  # [128, ch] bf16 (rel ids <= 63 exact)
        # merge: row = [xs_row (8*257 bf16) | xt_row (2*8*128 fp8)], then
        # partition-major [p, g8, row] so one DMA can cover several groups
        # with a single long contiguous descriptor per partition
        xm = np.concatenate(
            [xs_t.view(np.uint8), xt_t.view(np.uint8)], axis=1
        )
        rowb = xm.shape[1]
        xm = np.ascontiguousarray(
            xm.reshape(n_g8, P, rowb).transpose(1, 0, 2).reshape(P, n_g8 * rowb)
        )
        in_maps.append(
            {
                "xm": xm,
                "blr": blr,
                "w1": w1dr,
                "b1": b1_f,
                "w2": w2_bf,
                "io8": io8,
            }
        )
    return in_maps


# ------------------------------------------------------------- bass program

def build_bass(ch, cpb):
    """Build the SPMD-uniform per-core program."""
    nc = bacc.Bacc(
        "TRN2",
        target_bir_lowering=False,
        debug=False,
        num_devices=NCORES,
    )
    n_g8 = ch // 8
    ROWB = 8 * (D + 1) * 2 + 2 * 8 * P   # merged row bytes
    XSB = 8 * (D + 1) * 2
    xm = nc.dram_tensor("xm", [P, n_g8 * ROWB], mybir.dt.uint8,
                        kind="ExternalInput").ap()
    blr = nc.dram_tensor("blr", [P, ch], BF16, kind="ExternalInput").ap()
    w1 = nc.dram_tensor("w1", [P, 2 * H], F8, kind="ExternalInput").ap()
    b1 = nc.dram_tensor("b1", [H, 1], F32, kind="ExternalInput").ap()
    w2 = nc.dram_tensor("w2", [H, 1], BF16, kind="ExternalInput").ap()
    io8 = nc.dram_tensor("io8", [P, 8 * BLKG], BF16, kind="ExternalInput").ap()
    out = nc.dram_tensor("out", [GPC, D], F32, kind="ExternalOutput").ap()

    DR = mybir.MatmulPerfMode.DoubleRow

    with tile.TileContext(nc) as tc:
        with (
            tc.tile_pool(name="consts", bufs=1) as cpool,
            tc.tile_pool(name="xb", bufs=5) as xbpool,
            tc.tile_pool(name="hsb", bufs=4) as hsbpool,
            tc.tile_pool(name="e8", bufs=2) as epool,
            tc.tile_pool(name="oh", bufs=3) as ohpool,
            tc.tile_pool(name="outsb", bufs=2) as outpool,
            tc.tile_pool(name="acc", bufs=1, space="PSUM") as accpool,
            tc.tile_pool(name="hps", bufs=2, space="PSUM") as hpool,
            tc.tile_pool(name="lg", bufs=2, space="PSUM") as lgpool,
        ):
            # ---- constants into SBUF
            w1_sb = cpool.tile([P, 2 * H], F8, tag="w1")
            b1_sb = cpool.tile([H, 1], F32, tag="b1")
            w2_sb = cpool.tile([H, 1], BF16, tag="w2")
            io_sb = cpool.tile([P, 8 * BLKG], BF16, tag="io8")
            blr_sb = cpool.tile([P, ch], BF16, tag="blr")
            nc.sync.dma_start(w1_sb[:], w1[:])
            nc.sync.dma_start(b1_sb[:], b1[:])
            nc.sync.dma_start(w2_sb[:], w2[:])
            nc.sync.dma_start(io_sb[:], io8[:])
            nc.sync.dma_start(blr_sb[:], blr[:])
            w1_ap = w1_sb[:].rearrange("p (t m) -> p t m", t=2)

            # ---- persistent accumulators (PSUM)
            # col 0..255 pooled, col 256 denominator; two 64-graph blocks
            # pack into each [128, 257] f32 tile via partition halves
            pp = [
                accpool.tile([P, D + 1], F32, tag=f"pp{t}", name=f"pp{t}")
                for t in range(4)
            ]

            def pooled_out(b):
                r0 = (b % 2) * BLKG
                return pp[b // 2][r0 : r0 + BLKG, :]

            def flush_one(item, lg_thunk=None):
                # pooled[g, 0:256] += oh.T @ x ; den = col 256 (x ones col).
                oh_ap, xb, j, c = item
                b = c // cpb
                first = c == b * cpb
                last = c == (b + 1) * cpb - 1
                W = D + 1
                nc.tensor.matmul(
                    pooled_out(b),
                    oh_ap,
                    xb[:, j * W : (j + 1) * W],
                    start=first,
                    stop=last,
                )
                if lg_thunk is not None:
                    lg_thunk()

            # software pipeline: per iteration the PE stream is
            #   [DR(g), pool-flush(g-1) x8, logits(g) x8]
            # so the pooled burst of the previous group covers the tanh
            # latency of this one; is_eq (constants only) is hoisted off
            # the exp critical path.
            xmt = None
            prev = None          # (oh8, xb, c0) of previous group
            for g8 in range(n_g8):
                if g8 % 2 == 0:
                    xmt = xbpool.tile([P, 2 * ROWB], mybir.dt.uint8)
                    nc.sync.dma_start(
                        xmt[:], xm[:, g8 * ROWB : (g8 + 2) * ROWB]
                    )
                base = (g8 % 2) * ROWB
                xb = xmt[:, base : base + XSB].bitcast(BF16)  # [128, 8*257]
                xt_ap = (
                    xmt[:, base + XSB : base + ROWB]
                    .bitcast(F8)
                    .rearrange("p (t n) -> p t n", t=2)
                )  # [128, 2, 8*128]
                lg = lgpool.tile([P, 8], F32)
                hsbs = []
                # hT: fp8 DoubleRow, batches of 4 chunks
                for j0 in range(0, 8, 4):
                    hps = hpool.tile([P, 4 * P], F32)
                    nc.tensor.matmul(
                        hps[:],
                        w1_ap,
                        xt_ap[:, :, j0 * P : (j0 + 4) * P],
                        start=True,
                        stop=True,
                        perf_mode=DR,
                    )
                    hsb = hsbpool.tile([P, 4 * P], BF16)
                    nc.scalar.activation(
                        hsb[:], hps[:],
                        mybir.ActivationFunctionType.Tanh, bias=b1_sb[:],
                    )
                    hsbs.append(hsb)
                # indicator for this group: depends only on constants
                c0 = g8 * 8
                ind8 = ohpool.tile([P, 8 * BLKG], BF16, tag="ind8")
                io_v = io_sb[:].rearrange("p (j g) -> p j g", j=8)
                blr_v = blr_sb[:, c0 : c0 + 8].unsqueeze(-1).broadcast_to(
                    [P, 8, BLKG]
                )
                ind8_v = ind8[:].rearrange("p (j g) -> p j g", j=8)
                nc.vector.tensor_tensor(
                    ind8_v, io_v, blr_v, mybir.AluOpType.is_equal
                )
                # previous group's pooled flush (oh8/xb ready) keeps the
                # PE busy while this group's tanh runs on Scalar
                if prev is not None:
                    po, pxb, pc0 = prev
                    for j in range(8):
                        flush_one(
                            (po[:, j * BLKG : (j + 1) * BLKG], pxb, j,
                             pc0 + j)
                        )
                # logits for this group
                for j in range(8):
                    nc.tensor.matmul(
                        lg[:, j : j + 1],
                        hsbs[j // 4][:, (j % 4) * P : (j % 4 + 1) * P],
                        w2_sb[:],
                        start=True,
                        stop=True,
                    )
                e8 = epool.tile([P, 8], BF16)
                nc.scalar.activation(
                    e8[:], lg[:, 0:8], mybir.ActivationFunctionType.Exp
                )
                # oh[i, j*64+g] = ind8 * e[i, j]
                oh8 = ohpool.tile([P, 8 * BLKG], BF16, tag="oh8")
                e8_v = e8[:].unsqueeze(-1).broadcast_to([P, 8, BLKG])
                oh8_v = oh8[:].rearrange("p (j g) -> p j g", j=8)
                nc.vector.tensor_tensor(
                    oh8_v, ind8_v, e8_v, mybir.AluOpType.mult
                )
                prev = (oh8, xb, c0)
            po, pxb, pc0 = prev
            for j in range(8):
                flush_one(
                    (po[:, j * BLKG : (j + 1) * BLKG], pxb, j, pc0 + j)
                )

            # ---- epilogue: out[g] = pooled[g] / max(denom[g], tiny)
            recs = []
            for b in range(NBLK):
                acc = pooled_out(b)
                dmax = outpool.tile([BLKG, 1], F32, tag=f"dmax{b}", name=f"dmax{b}")
                rec = outpool.tile([BLKG, 1], F32, tag=f"rec{b}", name=f"rec{b}")
                nc.vector.tensor_scalar_max(dmax[:], acc[:, D : D + 1], 1e-30)
                nc.vector.reciprocal(rec[:], dmax[:])
                recs.append(rec)
            for b in range(NBLK):
                osb = outpool.tile([BLKG, D], F32, tag="osb")
                nc.scalar.mul(osb[:], pooled_out(b)[:, 0:D], recs[b][:])
                nc.sync.dma_start(out[b * BLKG : (b + 1) * BLKG, :], osb[:])

    nc.compile()
    return nc


# ----------------------------------------------------------------- kernel()

def kernel(**inputs):
    global LAST_RESULT
    x = np.asarray(inputs["x"])
    batch = np.asarray(inputs["batch"])
    W1 = np.asarray(inputs["W1"])
    b1 = np.asarray(inputs["b1"])
    W2 = np.asarray(inputs["W2"])
    # b2 cancels in the softmax; unused.

    seg, cpb, ch = make_plan(batch)
    in_maps = build_inputs(x, batch, W1, b1, W2, seg, cpb, ch)
    nc = build_bass(ch, cpb)
    res = bass_utils.run_bass_kernel_spmd(
        nc, in_maps, list(range(NCORES))
    )
    LAST_RESULT = res
    out = np.concatenate(
        [np.asarray(res.results[c]["out"]) for c in range(NCORES)], axis=0
    )
    return out.astype(np.float32)
